# revision 10
# baseline (speedup 1.0000x reference)
"""Causal self-attention on 8 TRN2 NeuronCores.

Sharding: head-parallel x batch-parallel. Core (b, g) computes heads
[8g, 8g+8) of batch b: qkv projection for those heads, causal attention,
and a partial output projection (rows 512g..512g+512 of W_proj). The host
sums the two partials per batch (tiny all-reduce done on host) and adds
b_proj.

Device layout is feature-major throughout (no on-device transposes):
  x^T (C x T) is pre-transposed on host. Q^T/K^T come out of the qkv
  matmul feature-major; V comes out token-major with a ones column
  appended so the attention row-sums fall out of the same PV matmul.
  Scores are S^T = K Q^T (key dim on partitions), so softmax needs no
  partition reductions: exp (no max-sub; scores are ~N(0,1)), triangle
  mask on diagonal 128-blocks, PV accumulate, then scale by 1/rowsum.
  The projection emits out^T which the host transposes back.
"""

import sys

sys.path.insert(0, "/opt/trn_rl_repo")

import numpy as np

import concourse.bass as bass  # noqa: F401
import concourse.tile as tile
from concourse import bacc, mybir
from concourse.bass_utils import run_bass_kernel_spmd

B, T, C = 4, 2048, 1024
H, D = 16, 64
G = 2           # head groups (cores per batch)
HPG = H // G    # heads per core = 8
PAIRS = HPG // 2
CH = T // 512   # 4 t-chunks of 512
ST = T // 128   # 16 s-tiles of 128
KO = C // 128   # 8 contraction tiles

F32 = mybir.dt.float32
F32R = mybir.dt.float32r
EXP = mybir.ActivationFunctionType.Exp

_CACHED_NC = None


def _build():
    nc = bacc.Bacc("TRN2", target_bir_lowering=False, debug=False)
    xT = nc.dram_tensor("xT", [C, T], F32R, kind="ExternalInput").ap()
    wqk = nc.dram_tensor("wqk", [PAIRS, 128, KO, 2, 128], F32R,
                         kind="ExternalInput").ap()
    wv = nc.dram_tensor("wv", [128, KO, 512], F32R, kind="ExternalInput").ap()
    wp = nc.dram_tensor("wp", [128, PAIRS, 8, 128], F32R,
                        kind="ExternalInput").ap()
    bqk = nc.dram_tensor("bqk", [128, 2, PAIRS], F32, kind="ExternalInput").ap()
    bv = nc.dram_tensor("bv", [1, 512], F32, kind="ExternalInput").ap()
    mask = nc.dram_tensor("mask", [128, 128], F32R, kind="ExternalInput").ap()
    vones = nc.dram_tensor("vones", [128, HPG], F32R, kind="ExternalInput").ap()
    out = nc.dram_tensor("out", [C, T], F32, kind="ExternalOutput").ap()

    with tile.TileContext(nc) as tc:
        with tc.tile_pool(name="persist", bufs=1) as pp:
            v_sb = [pp.tile([128, HPG, 65], F32R, name=f"v{i}", tag=f"v{i}") for i in range(ST)]
            qt = [pp.tile([128, T], F32R, name=f"qt{p}", tag=f"q{p}") for p in range(PAIRS)]
            kt = [pp.tile([128, T], F32R, name=f"kt{p}", tag=f"k{p}") for p in range(PAIRS)]

            # ---------------- phase A: qkv projection ----------------
            with tc.tile_pool(name="phA", bufs=1) as pa, \
                 tc.tile_pool(name="phA_wqk", bufs=2) as paw, \
                 tc.tile_pool(name="psA", bufs=4, space="PSUM") as psa:
                xt_sb = pa.tile([128, KO, T], F32R)
                nc.sync.dma_start(
                    xt_sb[:], xT.rearrange("(ko ki) t -> ki ko t", ki=128))
                wv_sb = pa.tile([128, KO, 512], F32R)
                nc.sync.dma_start(wv_sb[:], wv)
                bqk_sb = pa.tile([128, 2, PAIRS], F32)
                nc.sync.dma_start(bqk_sb[:], bqk)
                bv_sb = pa.tile([1, 512], F32)
                nc.sync.dma_start(bv_sb[:], bv)
                bv_bc = pa.tile([128, 512], F32)
                nc.gpsimd.partition_broadcast(bv_bc[:], bv_sb[0:1, :])

                # V (token-major) for all 8 heads + ones column
                for si in range(ST):
                    ps = psa.tile([128, 512], F32, tag="psA")
                    for ko in range(KO):
                        nc.tensor.matmul(
                            ps[:], xt_sb[:, ko, 128 * si:128 * si + 128],
                            wv_sb[:, ko, :],
                            start=(ko == 0), stop=(ko == KO - 1))
                    nc.vector.tensor_add(
                        v_sb[si][:, :, 0:64],
                        ps[:].rearrange("s (h d) -> s h d", d=64),
                        bv_bc[:].rearrange("s (h d) -> s h d", d=64))
                    nc.sync.dma_start(v_sb[si][:, :, 64:65], vones)

                # Q^T / K^T (feature-major) per head pair
                for p in range(PAIRS):
                    wqk_sb = paw.tile([128, KO, 2, 128], F32R, tag="wqk")
                    nc.sync.dma_start(wqk_sb[:], wqk[p])
                    for t, dst in ((0, qt[p]), (1, kt[p])):
                        for j in range(CH):
                            ps = psa.tile([128, 512], F32, tag="psA")
                            for ko in range(KO):
                                nc.tensor.matmul(
                                    ps[:], wqk_sb[:, ko, t, :],
                                    xt_sb[:, ko, 512 * j:512 * j + 512],
                                    start=(ko == 0), stop=(ko == KO - 1))
                            nc.vector.tensor_scalar_add(
                                dst[:, 512 * j:512 * j + 512], ps[:],
                                bqk_sb[:, t, p:p + 1])

            # ---------------- phases B+C ----------------
            with tc.tile_pool(name="phBC", bufs=1) as pbc:
                y2t = [pbc.tile([128, T], F32R, name=f"y2t{p}", tag=f"y{p}") for p in range(PAIRS)]
                wp_sb = pbc.tile([128, PAIRS, 8, 128], F32R)
                nc.sync.dma_start(wp_sb[:], wp)
                mask_sb = pbc.tile([128, 128], F32R)
                nc.sync.dma_start(mask_sb[:], mask)

                # ---------- phase B: attention ----------
                with tc.tile_pool(name="phB_p", bufs=3) as pb, \
                     tc.tile_pool(name="phB_r", bufs=2) as pr, \
                     tc.tile_pool(name="psS", bufs=3, space="PSUM") as pss, \
                     tc.tile_pool(name="psY", bufs=2, space="PSUM") as psy:
                    for p in range(PAIRS):
                        for j in range(CH):
                            n_tiles = 4 * j + 4
                            yps = [psy.tile([65, 512], F32, name=f"yps{h}", tag="Y")
                                   for h in range(2)]
                            for g0 in range(0, n_tiles, 2):
                                sps = [pss.tile([128, 1024], F32, name=f"sps{h}", tag="S")
                                       for h in range(2)]
                                pt = [pb.tile([128, 1024], F32R, name=f"pt{h}", tag="P")
                                      for h in range(2)]
                                offs = [max(0, 128 * (g0 + u) - 512 * j)
                                        for u in range(2)]
                                for h in range(2):  # head in pair
                                    lo, hi = 64 * h, 64 * h + 64
                                    for u in range(2):
                                        i = g0 + u
                                        off = offs[u]
                                        nc.tensor.matmul(
                                            sps[h][:, 512 * u + off:512 * (u + 1)],
                                            kt[p][lo:hi, 128 * i:128 * i + 128],
                                            qt[p][lo:hi,
                                                  512 * j + off:512 * (j + 1)],
                                            start=True, stop=True)
                                    nc.scalar.activation(
                                        pt[h][:, offs[0]:1024],
                                        sps[h][:, offs[0]:1024], EXP)
                                    for u in range(2):
                                        i = g0 + u
                                        off = offs[u]
                                        if i >= 4 * j:  # diagonal 128-block
                                            dlo = 512 * u + off
                                            nc.vector.tensor_mul(
                                                pt[h][:, dlo:dlo + 128],
                                                pt[h][:, dlo:dlo + 128],
                                                mask_sb[:])
                                        nc.tensor.matmul(
                                            yps[h][:, off:512],
                                            v_sb[i][:, 2 * p + h, :],
                                            pt[h][:, 512 * u + off:512 * (u + 1)],
                                            start=(i == 0),
                                            stop=(i == n_tiles - 1))
                            for h in range(2):
                                # psum rows 0:64 = y, row 64 = softmax sums
                                # (ones col in V_aug). DVE lanes can't cross
                                # partitions and PSUM bases must be
                                # 32-aligned, so: copy sums out at partition
                                # 64 (aligned), DMA-shift to partition 0,
                                # recip, gpsimd-broadcast, aligned mul, and
                                # DMA the normalized y into its y2t band.
                                st = pr.tile([65, 512], F32, tag="st")
                                nc.vector.tensor_copy(
                                    st[64:65, :], yps[h][64:65, :])
                                s0 = pr.tile([1, 512], F32, tag="s0")
                                nc.sync.dma_start(s0[:], st[64:65, :])
                                r0 = pr.tile([1, 512], F32, tag="r0")
                                nc.vector.reciprocal(r0[:], s0[:])
                                rb = pr.tile([64, 512], F32, tag="rb")
                                nc.gpsimd.partition_broadcast(rb[:], r0[:])
                                yn = pr.tile([64, 512], F32R, tag="yn")
                                nc.vector.tensor_mul(
                                    yn[:], yps[h][0:64, :], rb[:])
                                nc.sync.dma_start(
                                    y2t[p][64 * h:64 * h + 64,
                                           512 * j:512 * (j + 1)], yn[:])

                # ---------- phase C: output projection ----------
                with tc.tile_pool(name="phC", bufs=2) as pc, \
                     tc.tile_pool(name="psC", bufs=2, space="PSUM") as psc:
                    for o in range(8):
                        for j in range(CH):
                            ps = psc.tile([128, 512], F32, tag="psC")
                            for p in range(PAIRS):
                                nc.tensor.matmul(
                                    ps[:], wp_sb[:, p, o, :],
                                    y2t[p][:, 512 * j:512 * (j + 1)],
                                    start=(p == 0), stop=(p == PAIRS - 1))
                            ob = pc.tile([128, 512], F32, tag="ob")
                            nc.vector.tensor_copy(ob[:], ps[:])
                            nc.sync.dma_start(
                                out[128 * o:128 * o + 128,
                                    512 * j:512 * (j + 1)], ob[:])
    nc.compile()
    return nc


def _get_nc():
    global _CACHED_NC
    if _CACHED_NC is None:
        _CACHED_NC = _build()
    return _CACHED_NC


def _prep_in_maps(x, W_qkv, b_qkv, W_proj, b_proj):
    x = np.asarray(x, dtype=np.float32)
    W_qkv = np.asarray(W_qkv, dtype=np.float32)
    b_qkv = np.asarray(b_qkv, dtype=np.float32)
    W_proj = np.asarray(W_proj, dtype=np.float32)
    scale = np.float32(1.0 / np.sqrt(D))
    mask = np.triu(np.ones((128, 128), dtype=np.float32))

    per_g = []
    for g in range(G):
        cs, ce = 512 * g, 512 * g + 512
        Wq = W_qkv[:, cs:ce] * scale
        Wk = W_qkv[:, C + cs:C + ce]
        Wv = W_qkv[:, 2 * C + cs:2 * C + ce]
        # wqk[p, ki, ko, t, m] = W_t[128*ko + ki, 128*p + m]
        qk = np.stack([Wq, Wk], axis=0)  # (2, C, 512)
        qk = qk.reshape(2, KO, 128, PAIRS, 128)
        wqk = np.ascontiguousarray(qk.transpose(3, 2, 1, 0, 4))
        wv_b = np.ascontiguousarray(
            Wv.reshape(KO, 128, 512).transpose(1, 0, 2))
        # wp[ki, p, o, m] = W_proj[512*g + 128*p + ki, 128*o + m]
        wp_b = np.ascontiguousarray(
            W_proj[cs:ce].reshape(PAIRS, 128, 8, 128).transpose(1, 0, 2, 3))
        bq = b_qkv[cs:ce] * scale
        bk = b_qkv[C + cs:C + ce]
        # bqk[ki, t, p] = b_t[128*p + ki]
        bqk_b = np.ascontiguousarray(
            np.stack([bq, bk], 0).reshape(2, PAIRS, 128).transpose(2, 0, 1))
        bv_b = np.ascontiguousarray(b_qkv[2 * C + cs:2 * C + ce].reshape(1, 512))
        per_g.append(dict(wqk=wqk, wv=wv_b, wp=wp_b, bqk=bqk_b, bv=bv_b,
                          mask=mask, vones=np.ones((128, HPG), np.float32)))

    in_maps = []
    for b in range(B):
        xTb = np.ascontiguousarray(x[b].T)
        for g in range(G):
            in_maps.append({"xT": xTb, **per_g[g]})
    return in_maps


def kernel(x, W_qkv, b_qkv, W_proj, b_proj):
    nc = _get_nc()
    in_maps = _prep_in_maps(x, W_qkv, b_qkv, W_proj, b_proj)
    res = run_bass_kernel_spmd(nc, in_maps, core_ids=list(range(8)))
    b_proj = np.asarray(b_proj, dtype=np.float32)
    out = np.empty((B, T, C), dtype=np.float32)
    for b in range(B):
        acc = res.results[2 * b]["out"] + res.results[2 * b + 1]["out"]
        out[b] = acc.T + b_proj
    return out
